# revision 1
# baseline (speedup 1.0000x reference)
"""RNN-T JointNet kernel for 8 Trainium2 NeuronCores.

out[b,t,u,:] = gelu_tanh(enc[b,t]@We + dec[b,u]@Wd + b1) @ Wfc

Sharding: flatten (B=4, T=512) -> 2048 rows, 256 contiguous rows per core.
Core c handles batch b=c//2, time slice t0=(c%2)*256 .. +256. Each core only
needs its own enc slice and one batch's dec.

Per-core layout (all fp32):
  - encT (D=256, TC=256), decT (D=256, U=128): host-transposed so the first
    matmuls produce pe/pd with H on partitions, t/u on the free dim.
  - peb[h, t] = enc@We + b1 (4 h-tiles of (128, 256) in SBUF)
  - pd[h, u]  = dec@Wd      (4 h-tiles of (128, 128) in SBUF)
  - main loop over groups of TB=8 t's:
      DVE:  tmp[h, tb, u] = pd[h, u] + peb[h, t0+tb]   (broadcast APs)
      ACT:  hact = gelu_tanh(tmp)
      PE :  out_psum(u=128, v=512) = sum_ht hact[ht][:, tb]ᵀ-block @ Wfc[ht]
            (hact tile is the stationary operand, Wfc streams, N=512)
      DMA:  out_psum -> out[t] (contiguous 256 KiB)
"""

import sys

import numpy as np

sys.path.insert(0, "/opt/trn_rl_repo")

import concourse.bacc as bacc
import concourse.bass as bass
import concourse.mybir as mybir
import concourse.tile as tile
from concourse.bass_utils import run_bass_kernel_spmd

B, T, U, D, H, V = 4, 512, 128, 256, 512, 512
NCORES = 8
TC = (B * T) // NCORES  # 256 t-rows per core
TB = 8  # t's per main-loop group

_PROGRAM = None
LAST_RESULT = None


def _build():
    global _PROGRAM
    if _PROGRAM is not None:
        return _PROGRAM

    f32 = mybir.dt.float32
    # Bacc (not raw Bass): its compile() pipeline moves matmul waits onto
    # ldweights and splits >1-wait instructions via event semaphores —
    # walrus rejects matmuls carrying 2 sync waits otherwise.
    nc = bacc.Bacc("TRN2", target_bir_lowering=False)

    encT_d = nc.declare_dram_parameter("encT", (D, TC), f32, isOutput=False)
    decT_d = nc.declare_dram_parameter("decT", (D, U), f32, isOutput=False)
    w1_d = nc.declare_dram_parameter("W1", (2 * D, H), f32, isOutput=False)
    b1_d = nc.declare_dram_parameter("b1", (H, 1), f32, isOutput=False)
    wfc_d = nc.declare_dram_parameter("Wfc", (H, V), f32, isOutput=False)
    out_d = nc.declare_dram_parameter("out", (TC, U, V), f32, isOutput=True)

    GELU = mybir.ActivationFunctionType.Gelu_apprx_tanh
    IDENT = mybir.ActivationFunctionType.Identity

    with tile.TileContext(nc) as tc:
        with (
            tc.tile_pool(name="const", bufs=1) as cpool,
            tc.tile_pool(name="work", bufs=2) as wpool,
            tc.tile_pool(name="outsb", bufs=6) as osb_pool,
            tc.tile_pool(name="pro_ps", bufs=2, space="PSUM") as pro_ps,
            tc.tile_pool(name="out_ps", bufs=4, space="PSUM") as out_ps_pool,
        ):
            # W1 row-block i (128 rows of the 512-row input dim) lives at
            # cols [i*H, (i+1)*H). Blocks 0,1 = We; blocks 2,3 = Wd.
            w1_sb = cpool.tile([128, 4 * H], f32)
            wfc_sb = cpool.tile([128, 4 * V], f32)  # block ht = Wfc[ht*128:...]
            b1_sb = cpool.tile([128, 4], f32)  # col ht = b1[ht*128:(ht+1)*128]
            encT_sb = cpool.tile([128, 2 * TC], f32)
            decT_sb = cpool.tile([128, 2 * U], f32)
            peb_sb = cpool.tile([128, 4 * TC], f32)
            pd_sb = cpool.tile([128, 4 * U], f32)

            # One DMA per SBUF tile (3D APs): keeps the per-instruction sync
            # wait count low (walrus rejects >N waits on a matmul) and the
            # transfers large.
            nc.sync.dma_start(
                w1_sb.rearrange("p (i h) -> p i h", i=4),
                w1_d[:, :].rearrange("(i p) h -> p i h", p=128),
            )
            nc.sync.dma_start(
                wfc_sb.rearrange("p (i v) -> p i v", i=4),
                wfc_d[:, :].rearrange("(i p) v -> p i v", p=128),
            )
            nc.sync.dma_start(
                b1_sb, b1_d[:, :].rearrange("(i p) o -> p (i o)", p=128)
            )
            nc.sync.dma_start(
                encT_sb.rearrange("p (i t) -> p i t", i=2),
                encT_d[:, :].rearrange("(i p) t -> p i t", p=128),
            )
            nc.sync.dma_start(
                decT_sb.rearrange("p (i u) -> p i u", i=2),
                decT_d[:, :].rearrange("(i p) u -> p i u", p=128),
            )

            # Prologue: pe[h,t] = enc@We ; pdb[h,u] = dec@Wd + b1
            for ht in range(4):
                pe_ps = pro_ps.tile([128, TC], f32)
                for di in range(2):
                    nc.tensor.matmul(
                        pe_ps,
                        w1_sb[:, di * H + ht * 128 : di * H + (ht + 1) * 128],
                        encT_sb[:, di * TC : (di + 1) * TC],
                        start=(di == 0),
                        stop=(di == 1),
                    )
                nc.scalar.copy(peb_sb[:, ht * TC : (ht + 1) * TC], pe_ps)
                pd_ps = pro_ps.tile([128, U], f32)
                for di in range(2):
                    nc.tensor.matmul(
                        pd_ps,
                        w1_sb[:, (2 + di) * H + ht * 128 : (2 + di) * H + (ht + 1) * 128],
                        decT_sb[:, di * U : (di + 1) * U],
                        start=(di == 0),
                        stop=(di == 1),
                    )
                nc.scalar.activation(
                    pd_sb[:, ht * U : (ht + 1) * U],
                    pd_ps,
                    IDENT,
                    bias=b1_sb[:, ht : ht + 1],
                )

            # Main loop over u: ACT fuses the pd[:,u] add into the GELU as a
            # per-partition bias; h_u (h on partitions, t free) feeds the PE
            # as the stationary operand; DVE bounces PSUM->SBUF; DMA stores
            # (128 t, 512 v) slabs at fixed u.
            for u in range(U):
                hts = []
                for ht in range(4):
                    hact = wpool.tile([128, TC], f32, tag=f"h{ht}")
                    nc.scalar.activation(
                        hact,
                        peb_sb[:, ht * TC : (ht + 1) * TC],
                        GELU,
                        bias=pd_sb[:, ht * U + u : ht * U + u + 1],
                    )
                    hts.append(hact)
                for ts in range(TC // 128):
                    ops = out_ps_pool.tile([128, V], f32)
                    for ht in range(4):
                        nc.tensor.matmul(
                            ops,
                            hts[ht][:, ts * 128 : (ts + 1) * 128],
                            wfc_sb[:, ht * V : (ht + 1) * V],
                            start=(ht == 0),
                            stop=(ht == 3),
                        )
                    osb = osb_pool.tile([128, V], f32)
                    nc.vector.tensor_copy(osb, ops)
                    nc.sync.dma_start(
                        out_d[ts * 128 : (ts + 1) * 128, u : u + 1, :],
                        osb[:, None, :],
                    )

    nc.compile()
    _PROGRAM = nc
    return nc


def kernel(enc, dec, W1, b1, Wfc):
    global LAST_RESULT
    nc = _build()
    enc = np.asarray(enc, dtype=np.float32)
    dec = np.asarray(dec, dtype=np.float32)
    W1 = np.ascontiguousarray(np.asarray(W1, dtype=np.float32))
    b1 = np.ascontiguousarray(np.asarray(b1, dtype=np.float32).reshape(H, 1))
    Wfc = np.ascontiguousarray(np.asarray(Wfc, dtype=np.float32))

    in_maps = []
    for c in range(NCORES):
        b, t0 = c // 2, (c % 2) * TC
        in_maps.append(
            {
                "encT": np.ascontiguousarray(enc[b, t0 : t0 + TC, :].T),
                "decT": np.ascontiguousarray(dec[b].T),
                "W1": W1,
                "b1": b1,
                "Wfc": Wfc,
            }
        )

    LAST_RESULT = run_bass_kernel_spmd(nc, in_maps, list(range(NCORES)))

    out = np.empty((B, T, U, V), np.float32)
    for c in range(NCORES):
        b, t0 = c // 2, (c % 2) * TC
        out[b, t0 : t0 + TC] = LAST_RESULT.results[c]["out"]
    return out



# revision 11
# speedup vs baseline: 3.3630x; 3.3630x over previous
"""RNN-T JointNet kernel for 8 Trainium2 NeuronCores.

out[b,t,u,:] = gelu_tanh(enc[b,t]@We + dec[b,u]@Wd + b1) @ Wfc

Sharding: flatten (B=4, T=512) -> 2048 rows, 256 contiguous rows per core.
Core c handles batch b=c//2, time slice t0=(c%2)*256 .. +256. Each core only
needs its own enc slice and one batch's dec.

Per-core layout (all fp32):
  - encT (D=256, TC=256), decT (D=256, U=128): host-transposed so the first
    matmuls produce pe/pd with H on partitions, t/u on the free dim.
  - peb[h, t] = enc@We + b1 (4 h-tiles of (128, 256) in SBUF)
  - pd[h, u]  = dec@Wd      (4 h-tiles of (128, 128) in SBUF)
  - main loop over groups of TB=8 t's:
      DVE:  tmp[h, tb, u] = pd[h, u] + peb[h, t0+tb]   (broadcast APs)
      ACT:  hact = gelu_tanh(tmp)
      PE :  out_psum(u=128, v=512) = sum_ht hact[ht][:, tb]ᵀ-block @ Wfc[ht]
            (hact tile is the stationary operand, Wfc streams, N=512)
      DMA:  out_psum -> out[t] (contiguous 256 KiB)
"""

import sys

import numpy as np

sys.path.insert(0, "/opt/trn_rl_repo")

import concourse.bacc as bacc
import concourse.bass as bass
import concourse.mybir as mybir
import concourse.tile as tile
from concourse.bass_utils import run_bass_kernel_spmd

B, T, U, D, H, V = 4, 512, 128, 256, 512, 512
NCORES = 8
TC = (B * T) // NCORES  # 256 t-rows per core
TB = 8  # t's per main-loop group

_PROGRAM = None
LAST_RESULT = None


def _build():
    global _PROGRAM
    if _PROGRAM is not None:
        return _PROGRAM

    f32 = mybir.dt.float32
    # Bacc (not raw Bass): its compile() pipeline moves matmul waits onto
    # ldweights and splits >1-wait instructions via event semaphores —
    # walrus rejects matmuls carrying 2 sync waits otherwise.
    nc = bacc.Bacc("TRN2", target_bir_lowering=False)

    encT_d = nc.declare_dram_parameter("encT", (D, TC), f32, isOutput=False)
    decT_d = nc.declare_dram_parameter("decT", (D, U), f32, isOutput=False)
    w1_d = nc.declare_dram_parameter("W1", (2 * D, H), f32, isOutput=False)
    b1_d = nc.declare_dram_parameter("b1", (H, 1), f32, isOutput=False)
    wfc_d = nc.declare_dram_parameter("Wfc", (H, V), f32, isOutput=False)
    out_d = nc.declare_dram_parameter("out", (TC, U, V), f32, isOutput=True)

    GELU = mybir.ActivationFunctionType.Gelu_apprx_tanh
    IDENT = mybir.ActivationFunctionType.Identity
    # Same bits as fp32, but the PE streams it at 1 cycle/row (vs 4 for
    # plain fp32) when the moving free dim is >= 256.
    F32R = mybir.dt.float32r

    with tile.TileContext(nc) as tc:
        with (
            tc.tile_pool(name="const", bufs=1) as cpool,
            tc.tile_pool(name="work", bufs=2) as wpool,
            tc.tile_pool(name="outsb", bufs=6) as osb_pool,
            tc.tile_pool(name="pro_ps", bufs=2, space="PSUM") as pro_ps,
            tc.tile_pool(name="out_ps", bufs=4, space="PSUM") as out_ps_pool,
        ):
            # W1 row-block i (128 rows of the 512-row input dim) lives at
            # cols [i*H, (i+1)*H). Blocks 0,1 = We; blocks 2,3 = Wd.
            # Tiles that only feed matmuls are float32r so the PE streams
            # them at 1 cycle/row; the BIR verifier requires the producer
            # (DMA/ACT) output dtype to be f32r as well.
            w1_sb = cpool.tile([128, 4 * H], F32R)
            wfc_sb = cpool.tile([128, 4 * V], F32R)  # block ht = Wfc[ht*128:...]
            b1_sb = cpool.tile([128, 4], f32)  # col ht = b1[ht*128:(ht+1)*128]
            encT_sb = cpool.tile([128, 2 * TC], F32R)
            decT_sb = cpool.tile([128, 2 * U], F32R)
            peb_sb = cpool.tile([128, 4 * TC], f32)
            pd_sb = cpool.tile([128, 4 * U], f32)

            # One DMA per SBUF tile (3D APs): keeps the per-instruction sync
            # wait count low (walrus rejects >N waits on a matmul) and the
            # transfers large.
            nc.sync.dma_start(
                w1_sb.rearrange("p (i h) -> p i h", i=4),
                w1_d[:, :].rearrange("(i p) h -> p i h", p=128).bitcast(F32R),
            )
            nc.sync.dma_start(
                wfc_sb.rearrange("p (i v) -> p i v", i=4),
                wfc_d[:, :].rearrange("(i p) v -> p i v", p=128).bitcast(F32R),
            )
            nc.sync.dma_start(
                b1_sb, b1_d[:, :].rearrange("(i p) o -> p (i o)", p=128)
            )
            nc.sync.dma_start(
                encT_sb.rearrange("p (i t) -> p i t", i=2),
                encT_d[:, :].rearrange("(i p) t -> p i t", p=128).bitcast(F32R),
            )
            nc.sync.dma_start(
                decT_sb.rearrange("p (i u) -> p i u", i=2),
                decT_d[:, :].rearrange("(i p) u -> p i u", p=128).bitcast(F32R),
            )

            # Prologue: pe[h,t] = enc@We ; pdb[h,u] = dec@Wd + b1
            for ht in range(4):
                pe_ps = pro_ps.tile([128, TC], f32)
                for di in range(2):
                    nc.tensor.matmul(
                        pe_ps,
                        w1_sb[:, di * H + ht * 128 : di * H + (ht + 1) * 128],
                        encT_sb[:, di * TC : (di + 1) * TC],
                        start=(di == 0),
                        stop=(di == 1),
                    )
                nc.scalar.copy(peb_sb[:, ht * TC : (ht + 1) * TC], pe_ps)
                pd_ps = pro_ps.tile([128, U], f32)
                for di in range(2):
                    nc.tensor.matmul(
                        pd_ps,
                        w1_sb[:, (2 + di) * H + ht * 128 : (2 + di) * H + (ht + 1) * 128],
                        decT_sb[:, di * U : (di + 1) * U],
                        start=(di == 0),
                        stop=(di == 1),
                    )
                nc.scalar.activation(
                    pd_sb[:, ht * U : (ht + 1) * U],
                    pd_ps,
                    IDENT,
                    bias=b1_sb[:, ht : ht + 1],
                )

            # Main loop over u: ACT fuses the pd[:,u] add into the GELU as a
            # per-partition bias; h_u (h on partitions, t free) feeds the PE
            # as the stationary operand; DVE bounces PSUM->SBUF; DMA stores
            # (128 t, 512 v) slabs at fixed u.
            for u in range(U):
                hts = []
                for ht in range(4):
                    hact = wpool.tile([128, TC], F32R, tag=f"h{ht}")
                    nc.scalar.activation(
                        hact,
                        peb_sb[:, ht * TC : (ht + 1) * TC],
                        GELU,
                        bias=pd_sb[:, ht * U + u : ht * U + u + 1],
                    )
                    hts.append(hact)
                for ts in range(TC // 128):
                    ops = out_ps_pool.tile([128, V], f32)
                    for ht in range(4):
                        nc.tensor.matmul(
                            ops,
                            hts[ht][:, ts * 128 : (ts + 1) * 128],
                            wfc_sb[:, ht * V : (ht + 1) * V],
                            start=(ht == 0),
                            stop=(ht == 3),
                        )
                    osb = osb_pool.tile([128, V], f32)
                    nc.vector.tensor_copy(osb, ops)
                    nc.sync.dma_start(
                        out_d[ts * 128 : (ts + 1) * 128, u : u + 1, :],
                        osb[:, None, :],
                    )

    nc.compile()
    _PROGRAM = nc
    return nc


def kernel(enc, dec, W1, b1, Wfc):
    global LAST_RESULT
    nc = _build()
    enc = np.asarray(enc, dtype=np.float32)
    dec = np.asarray(dec, dtype=np.float32)
    W1 = np.ascontiguousarray(np.asarray(W1, dtype=np.float32))
    b1 = np.ascontiguousarray(np.asarray(b1, dtype=np.float32).reshape(H, 1))
    Wfc = np.ascontiguousarray(np.asarray(Wfc, dtype=np.float32))

    in_maps = []
    for c in range(NCORES):
        b, t0 = c // 2, (c % 2) * TC
        in_maps.append(
            {
                "encT": np.ascontiguousarray(enc[b, t0 : t0 + TC, :].T),
                "decT": np.ascontiguousarray(dec[b].T),
                "W1": W1,
                "b1": b1,
                "Wfc": Wfc,
            }
        )

    LAST_RESULT = run_bass_kernel_spmd(nc, in_maps, list(range(NCORES)))

    out = np.empty((B, T, U, V), np.float32)
    for c in range(NCORES):
        b, t0 = c // 2, (c % 2) * TC
        out[b, t0 : t0 + TC] = LAST_RESULT.results[c]["out"]
    return out

